# revision 10
# baseline (speedup 1.0000x reference)
"""Trainium2 Bass kernel for nn_ContrastiveLoss (SimCLR-style, N=8192, D=128).

Sharding: rows of the NxN sim matrix split across 8 cores (1024 rows each).
Each core receives the full z = concat(emb0, emb1) ROTATED so its own rows
come first (np.roll(z, -core*1024, axis=0)).  With that rotation the diagonal
of row-block b sits at local columns [b*128, b*128+128) and the positive pair
at local columns [4096+b*128, ...), identical on every core -> one SPMD
program, no collectives.

Math (per row r, fixed max = 1.0 since cosine sim <= 1):
  e_j  = exp(10*G_rj - 10),  S_r = sum_j e_j - e_rr
  loss_r = lse_r - 10*G_pos = (10 + ln S_r) - (ln e_pos + 10) = ln S_r - ln e_pos
  loss   = mean_r(loss_r);  per-core output = [128,1] partial sums of loss_r.

Engine split per core: PE does z_blk @ z^T (bf16 operands, fp32 psum),
ACT does exp(10x-10) on each [128,2048] psum chunk, GPSIMD row-sums the
exp output, DVE does norms + diag/pos extraction.  znT is produced via
DMA-transpose (bf16, sync queue) straight from the normalized tiles.
"""

import sys

sys.path.insert(0, "/opt/trn_rl_repo")

from contextlib import ExitStack

import numpy as np

import concourse.bass as bass
import concourse.bacc as bacc
import concourse.tile as tile
from concourse import mybir
from concourse import bass_utils
from concourse.masks import make_identity

B = 4096
D = 128
N = 2 * B            # 8192 rows of z
NCORES = 8
ROWS = N // NCORES   # 1024 rows per core
NBLK = ROWS // 128   # 8 row-blocks per core
CHUNK = 2048         # psum tile width (4 banks)
NCHUNK = N // CHUNK  # 4 column chunks
SEG = 512            # matmul moving-operand width
NTILE = N // 128     # 64 partition-tiles of z
GRP = 8              # tiles per DMA / norm group
INV_T = 10.0         # 1/temperature
EPS = 1e-8

F32 = mybir.dt.float32
BF16 = mybir.dt.bfloat16
AX = mybir.AxisListType
AF = mybir.ActivationFunctionType


def _build() -> bass.Bass:
    nc = bacc.Bacc(None)
    z_in = nc.declare_dram_parameter("z", [N, D], F32, isOutput=False)
    out = nc.declare_dram_parameter("partial", [128, 1], F32, isOutput=True)

    z_re = z_in.rearrange("(n p) d -> p n d", p=128)  # row = n*128 + p

    with tile.TileContext(nc) as tc:
        with ExitStack() as ctx:
            persist = ctx.enter_context(tc.tile_pool(name="persist", bufs=1))
            work = ctx.enter_context(tc.tile_pool(name="work", bufs=3))
            junkp = ctx.enter_context(tc.tile_pool(name="junk", bufs=3))
            psum = ctx.enter_context(tc.tile_pool(name="psum", bufs=2, space="PSUM"))

            ident = persist.tile([128, 128], F32)
            make_identity(nc, ident)
            # non-Copy activations need bias as an SBUF AP
            b_zero = persist.tile([128, 1], F32)
            nc.vector.memset(b_zero, 0.0)
            b_neg10 = persist.tile([128, 1], F32)
            nc.vector.memset(b_neg10, -INV_T)

            # ---- load z (gpsimd queue) + per-group row norms -------------
            z_sb = persist.tile([128, NTILE, D], F32)
            sq = persist.tile([128, NTILE, D], F32)
            rn = persist.tile([128, NTILE], F32)
            for i in range(NTILE // GRP):
                sl = slice(i * GRP, (i + 1) * GRP)
                nc.gpsimd.dma_start(out=z_sb[:, sl, :], in_=z_re[:, sl, :])
                nc.vector.tensor_mul(sq[:, sl, :], z_sb[:, sl, :], z_sb[:, sl, :])
                nc.vector.reduce_sum(rn[:, sl], sq[:, sl, :], axis=AX.X)
                nc.scalar.activation(rn[:, sl], rn[:, sl], AF.Sqrt, bias=b_zero)
                nc.vector.tensor_scalar_max(rn[:, sl], rn[:, sl], EPS)
                nc.vector.reciprocal(rn[:, sl], rn[:, sl])

            # ---- normalize (bf16) + DMA-transpose into znT chunks --------
            znT = [
                persist.tile([128, CHUNK], BF16, tag=f"znT{j}", name=f"znT{j}")
                for j in range(NCHUNK)
            ]
            for n in range(NTILE):
                znsc = work.tile([128, 128], BF16, tag="znsc")
                nc.vector.tensor_scalar_mul(znsc, z_sb[:, n, :], rn[:, n : n + 1])
                j, k = divmod(n, CHUNK // 128)
                nc.sync.dma_start(
                    znT[j][:, k * 128 : (k + 1) * 128], znsc, transpose=True
                )

            # ---- main loop: sim row-blocks x column chunks ---------------
            acc = persist.tile([128, NBLK, NCHUNK], F32)   # per-chunk exp sums
            e_diag = persist.tile([128, NBLK], F32)
            e_pos = persist.tile([128, NBLK], F32)

            for b in range(NBLK):
                lhsT = znT[0][:, b * 128 : (b + 1) * 128]  # block cols < 1024
                for c in range(NCHUNK):
                    pt = psum.tile([128, CHUNK], F32, tag="pp")
                    for s in range(CHUNK // SEG):
                        nc.tensor.matmul(
                            pt[:, s * SEG : (s + 1) * SEG],
                            lhsT,
                            znT[c][:, s * SEG : (s + 1) * SEG],
                            start=True,
                            stop=True,
                        )
                    ej = junkp.tile([128, CHUNK], F32, tag="ej")
                    nc.scalar.activation(
                        ej, pt, AF.Exp, scale=INV_T, bias=b_neg10,
                        accum_out=acc[:, b, c : c + 1],
                    )
                    if c == 0:  # e_rr at cols b*128..+128 of chunk 0
                        scr = work.tile([128, 128], F32, tag="scr")
                        nc.vector.tensor_mul(scr, ej[:, b * 128 : b * 128 + 128], ident)
                        nc.vector.reduce_sum(e_diag[:, b : b + 1], scr, axis=AX.X)
                    if c == 2:  # e_pos at cols 4096 + b*128..+128
                        scr2 = work.tile([128, 128], F32, tag="scr2")
                        nc.vector.tensor_mul(scr2, ej[:, b * 128 : b * 128 + 128], ident)
                        nc.vector.reduce_sum(e_pos[:, b : b + 1], scr2, axis=AX.X)

            # ---- epilogue ------------------------------------------------
            sumexp = persist.tile([128, NBLK], F32)
            nc.vector.reduce_sum(sumexp, acc, axis=AX.X)      # [128,8,4] -> [128,8]
            S = persist.tile([128, NBLK], F32)
            nc.vector.tensor_sub(S, sumexp, e_diag)
            lnS = persist.tile([128, NBLK], F32)
            nc.scalar.activation(lnS, S, AF.Ln, bias=b_zero)
            lnp = persist.tile([128, NBLK], F32)
            nc.scalar.activation(lnp, e_pos, AF.Ln, bias=b_zero)
            contrib = persist.tile([128, NBLK], F32)
            nc.vector.tensor_sub(contrib, lnS, lnp)
            total = persist.tile([128, 1], F32)
            nc.vector.reduce_sum(total, contrib, axis=AX.X)
            nc.sync.dma_start(out=out[:, :], in_=total)

    nc.compile()
    return nc


_NC = None


def _get_nc() -> bass.Bass:
    global _NC
    if _NC is None:
        _NC = _build()
    return _NC


def kernel(emb0: np.ndarray, emb1: np.ndarray) -> np.ndarray:
    z = np.concatenate(
        [np.asarray(emb0, np.float32), np.asarray(emb1, np.float32)], axis=0
    )
    in_maps = [
        {"z": np.ascontiguousarray(np.roll(z, -c * ROWS, axis=0))}
        for c in range(NCORES)
    ]
    res = bass_utils.run_bass_kernel_spmd(_get_nc(), in_maps, core_ids=list(range(NCORES)))
    total = sum(float(r["partial"].sum(dtype=np.float64)) for r in res.results)
    return np.asarray(np.float32(total / N))


# revision 11
# speedup vs baseline: 1.3549x; 1.3549x over previous
"""Trainium2 Bass kernel for nn_ContrastiveLoss (SimCLR-style, N=8192, D=128).

Sharding: rows of the NxN sim matrix split across 8 cores (1024 rows each).
Each core receives the full z = concat(emb0, emb1) ROTATED so its own rows
come first (np.roll(z, -core*1024, axis=0)).  With that rotation the diagonal
of row-block b sits at local columns [b*128, b*128+128) and the positive pair
at local columns [4096+b*128, ...), identical on every core -> one SPMD
program, no collectives.

Math (per row r, fixed max = 1.0 since cosine sim <= 1):
  e_j  = exp(10*G_rj - 10),  S_r = sum_j e_j - e_rr
  loss_r = lse_r - 10*G_pos = (10 + ln S_r) - (ln e_pos + 10) = ln S_r - ln e_pos
  loss   = mean_r(loss_r);  per-core output = [128,1] partial sums of loss_r.

Engine split per core: PE does z_blk @ z^T (bf16 operands, fp32 psum)
plus the zn transposes; ACT does exp(10x-10) on each [128,2048] psum chunk
with accum_out row-sums; DVE does norms, psum->bf16 casts (batched 512 wide)
and diag/pos extraction from the exp output in SBUF.
"""

import sys

sys.path.insert(0, "/opt/trn_rl_repo")

from contextlib import ExitStack

import numpy as np

import concourse.bass as bass
import concourse.bacc as bacc
import concourse.tile as tile
from concourse import mybir
from concourse import bass_utils
from concourse.masks import make_identity

B = 4096
D = 128
N = 2 * B            # 8192 rows of z
NCORES = 8
ROWS = N // NCORES   # 1024 rows per core
NBLK = ROWS // 128   # 8 row-blocks per core
CHUNK = 2048         # psum tile width (4 banks)
NCHUNK = N // CHUNK  # 4 column chunks
SEG = 512            # matmul moving-operand width
NTILE = N // 128     # 64 partition-tiles of z
GRP = 8              # tiles per DMA / norm group
INV_T = 10.0         # 1/temperature
EPS = 1e-8

F32 = mybir.dt.float32
BF16 = mybir.dt.bfloat16
AX = mybir.AxisListType
AF = mybir.ActivationFunctionType


def _build() -> bass.Bass:
    nc = bacc.Bacc(None)
    z_in = nc.declare_dram_parameter("z", [N, D], F32, isOutput=False)
    out = nc.declare_dram_parameter("partial", [128, 1], F32, isOutput=True)

    z_re = z_in.rearrange("(n p) d -> p n d", p=128)  # row = n*128 + p

    with tile.TileContext(nc) as tc:
        with ExitStack() as ctx:
            persist = ctx.enter_context(tc.tile_pool(name="persist", bufs=1))
            work = ctx.enter_context(tc.tile_pool(name="work", bufs=3))
            junkp = ctx.enter_context(tc.tile_pool(name="junk", bufs=3))
            psum = ctx.enter_context(tc.tile_pool(name="psum", bufs=2, space="PSUM"))

            ident = persist.tile([128, 128], F32)
            make_identity(nc, ident)
            # non-Copy activations need bias as an SBUF AP
            b_zero = persist.tile([128, 1], F32)
            nc.vector.memset(b_zero, 0.0)
            b_neg10 = persist.tile([128, 1], F32)
            nc.vector.memset(b_neg10, -INV_T)

            # ---- load z (gpsimd queue) + per-group row norms -------------
            z_sb = persist.tile([128, NTILE, D], F32)
            sq = persist.tile([128, NTILE, D], F32)
            rn = persist.tile([128, NTILE], F32)
            for i in range(NTILE // GRP):
                sl = slice(i * GRP, (i + 1) * GRP)
                nc.sync.dma_start(out=z_sb[:, sl, :], in_=z_re[:, sl, :])
                nc.vector.tensor_mul(sq[:, sl, :], z_sb[:, sl, :], z_sb[:, sl, :])
                nc.vector.reduce_sum(rn[:, sl], sq[:, sl, :], axis=AX.X)
                nc.scalar.activation(rn[:, sl], rn[:, sl], AF.Sqrt, bias=b_zero)
                nc.vector.tensor_scalar_max(rn[:, sl], rn[:, sl], EPS)
                nc.vector.reciprocal(rn[:, sl], rn[:, sl])

            # ---- normalize (bf16) + DMA-transpose into znT chunks --------
            znT = [
                persist.tile([128, CHUNK], BF16, tag=f"znT{j}", name=f"znT{j}")
                for j in range(NCHUNK)
            ]
            for g in range(NTILE // 4):  # 4 tiles -> one [128,512] psum, one cast
                tp = psum.tile([128, 512], F32, tag="pp")
                for q in range(4):
                    n = g * 4 + q
                    znsc = work.tile([128, 128], F32, tag="znsc")
                    nc.vector.tensor_scalar_mul(znsc, z_sb[:, n, :], rn[:, n : n + 1])
                    nc.tensor.transpose(tp[:, q * 128 : (q + 1) * 128], znsc, ident)
                j, k = divmod(g * 4, CHUNK // 128)
                nc.vector.tensor_copy(znT[j][:, k * 128 : k * 128 + 512], tp)

            # ---- main loop: sim row-blocks x column chunks ---------------
            acc = persist.tile([128, NBLK, NCHUNK], F32)   # per-chunk exp sums
            e_diag = persist.tile([128, NBLK], F32)
            e_pos = persist.tile([128, NBLK], F32)

            for b in range(NBLK):
                lhsT = znT[0][:, b * 128 : (b + 1) * 128]  # block cols < 1024
                for c in range(NCHUNK):
                    pt = psum.tile([128, CHUNK], F32, tag="pp")
                    for s in range(CHUNK // SEG):
                        nc.tensor.matmul(
                            pt[:, s * SEG : (s + 1) * SEG],
                            lhsT,
                            znT[c][:, s * SEG : (s + 1) * SEG],
                            start=True,
                            stop=True,
                        )
                    ej = junkp.tile([128, CHUNK], F32, tag="ej")
                    nc.scalar.activation(
                        ej, pt, AF.Exp, scale=INV_T, bias=b_neg10,
                        accum_out=acc[:, b, c : c + 1],
                    )
                    if c == 0:  # e_rr at cols b*128..+128 of chunk 0
                        scr = work.tile([128, 128], F32, tag="scr")
                        nc.vector.tensor_mul(scr, ej[:, b * 128 : b * 128 + 128], ident)
                        nc.vector.reduce_sum(e_diag[:, b : b + 1], scr, axis=AX.X)
                    if c == 2:  # e_pos at cols 4096 + b*128..+128
                        scr2 = work.tile([128, 128], F32, tag="scr2")
                        nc.vector.tensor_mul(scr2, ej[:, b * 128 : b * 128 + 128], ident)
                        nc.vector.reduce_sum(e_pos[:, b : b + 1], scr2, axis=AX.X)

            # ---- epilogue ------------------------------------------------
            sumexp = persist.tile([128, NBLK], F32)
            nc.vector.reduce_sum(sumexp, acc, axis=AX.X)      # [128,8,4] -> [128,8]
            S = persist.tile([128, NBLK], F32)
            nc.vector.tensor_sub(S, sumexp, e_diag)
            lnS = persist.tile([128, NBLK], F32)
            nc.scalar.activation(lnS, S, AF.Ln, bias=b_zero)
            lnp = persist.tile([128, NBLK], F32)
            nc.scalar.activation(lnp, e_pos, AF.Ln, bias=b_zero)
            contrib = persist.tile([128, NBLK], F32)
            nc.vector.tensor_sub(contrib, lnS, lnp)
            total = persist.tile([128, 1], F32)
            nc.vector.reduce_sum(total, contrib, axis=AX.X)
            nc.sync.dma_start(out=out[:, :], in_=total)

    nc.compile()
    return nc


_NC = None


def _get_nc() -> bass.Bass:
    global _NC
    if _NC is None:
        _NC = _build()
    return _NC


def kernel(emb0: np.ndarray, emb1: np.ndarray) -> np.ndarray:
    z = np.concatenate(
        [np.asarray(emb0, np.float32), np.asarray(emb1, np.float32)], axis=0
    )
    in_maps = [
        {"z": np.ascontiguousarray(np.roll(z, -c * ROWS, axis=0))}
        for c in range(NCORES)
    ]
    res = bass_utils.run_bass_kernel_spmd(_get_nc(), in_maps, core_ids=list(range(NCORES)))
    total = sum(float(r["partial"].sum(dtype=np.float64)) for r in res.results)
    return np.asarray(np.float32(total / N))


# revision 13
# speedup vs baseline: 1.5237x; 1.1246x over previous
"""Trainium2 Bass kernel for nn_ContrastiveLoss (SimCLR-style, N=8192, D=128).

Sharding: rows of the NxN sim matrix split across 8 cores (1024 rows each).
Each core receives the full z = concat(emb0, emb1) ROTATED so its own rows
come first (np.roll(z, -core*1024, axis=0)).  With that rotation the diagonal
of row-block b sits at local columns [b*128, b*128+128) and the positive pair
at local columns [4096+b*128, ...), identical on every core -> one SPMD
program, no collectives.

Math (per row r, fixed max = 1.0 since cosine sim <= 1):
  e_j  = exp(10*G_rj - 10),  S_r = sum_j e_j - e_rr
  loss_r = lse_r - 10*G_pos = (10 + ln S_r) - (ln e_pos + 10) = ln S_r - ln e_pos
  loss   = mean_r(loss_r);  per-core output = [128,1] partial sums of loss_r.

Engine split per core: PE does z_blk @ z^T (bf16 operands, fp32 psum)
plus the zn transposes; ACT does exp(10x-10) on each [128,2048] psum chunk
with accum_out row-sums; DVE does norms, psum->bf16 casts (batched 512 wide)
and diag/pos extraction from the exp output in SBUF.
"""

import sys

sys.path.insert(0, "/opt/trn_rl_repo")

from contextlib import ExitStack

import numpy as np

import concourse.bass as bass
import concourse.bacc as bacc
import concourse.tile as tile
from concourse import mybir
from concourse import bass_utils
from concourse.masks import make_identity

B = 4096
D = 128
N = 2 * B            # 8192 rows of z
NCORES = 8
ROWS = N // NCORES   # 1024 rows per core
NBLK = ROWS // 128   # 8 row-blocks per core
CHUNK = 2048         # psum tile width (4 banks)
NCHUNK = N // CHUNK  # 4 column chunks
SEG = 512            # matmul moving-operand width
NTILE = N // 128     # 64 partition-tiles of z
GRP = 8              # tiles per DMA / norm group
INV_T = 10.0         # 1/temperature
EPS = 1e-8

F32 = mybir.dt.float32
BF16 = mybir.dt.bfloat16
AX = mybir.AxisListType
AF = mybir.ActivationFunctionType


def _build() -> bass.Bass:
    nc = bacc.Bacc(None)
    z_in = nc.declare_dram_parameter("z", [N, D], F32, isOutput=False)
    out = nc.declare_dram_parameter("partial", [128, 1], F32, isOutput=True)

    z_re = z_in.rearrange("(n p) d -> p n d", p=128)  # row = n*128 + p

    with tile.TileContext(nc) as tc:
        with ExitStack() as ctx:
            persist = ctx.enter_context(tc.tile_pool(name="persist", bufs=1))
            work = ctx.enter_context(tc.tile_pool(name="work", bufs=3))
            junkp = ctx.enter_context(tc.tile_pool(name="junk", bufs=3))
            psum = ctx.enter_context(tc.tile_pool(name="psum", bufs=2, space="PSUM"))

            ident = persist.tile([128, 128], BF16)
            make_identity(nc, ident)
            # non-Copy activations need bias as an SBUF AP
            b_zero = persist.tile([128, 1], F32)
            nc.vector.memset(b_zero, 0.0)
            b_neg10 = persist.tile([128, 1], F32)
            nc.vector.memset(b_neg10, -INV_T)

            # ---- load z + per-group row norms + bf16 normalize -----------
            z_sb = persist.tile([128, NTILE, D], F32)
            sq = persist.tile([128, NTILE, D], F32)
            rn = persist.tile([128, NTILE], F32)
            zn_all = persist.tile([128, NTILE, D], BF16)
            for i in range(NTILE // GRP):
                sl = slice(i * GRP, (i + 1) * GRP)
                nc.sync.dma_start(out=z_sb[:, sl, :], in_=z_re[:, sl, :])
                nc.vector.tensor_mul(sq[:, sl, :], z_sb[:, sl, :], z_sb[:, sl, :])
                nc.vector.reduce_sum(rn[:, sl], sq[:, sl, :], axis=AX.X)
                nc.scalar.activation(rn[:, sl], rn[:, sl], AF.Sqrt, bias=b_zero)
                nc.vector.tensor_scalar_max(rn[:, sl], rn[:, sl], EPS)
                nc.vector.reciprocal(rn[:, sl], rn[:, sl])
                nc.vector.tensor_mul(
                    zn_all[:, sl, :],
                    z_sb[:, sl, :],
                    rn[:, sl].broadcast_to((128, GRP, D)),
                )

            # ---- transpose into znT chunks (PE, 16 tiles per psum slot) --
            znT = [
                persist.tile([128, CHUNK], BF16, tag=f"znT{j}", name=f"znT{j}")
                for j in range(NCHUNK)
            ]
            for j in range(NCHUNK):
                tp = psum.tile([128, CHUNK], BF16, tag="pp")
                for q in range(CHUNK // 128):
                    n = j * (CHUNK // 128) + q
                    nc.tensor.transpose(
                        tp[:, q * 128 : (q + 1) * 128], zn_all[:, n, :], ident
                    )
                nc.vector.tensor_copy(znT[j], tp)

            # ---- main loop: sim row-blocks x column chunks ---------------
            acc = persist.tile([128, NBLK, NCHUNK], F32)   # per-chunk exp sums
            e_diag = persist.tile([128, NBLK], F32)
            e_pos = persist.tile([128, NBLK], F32)

            for b in range(NBLK):
                lhsT = znT[0][:, b * 128 : (b + 1) * 128]  # block cols < 1024
                for c in range(NCHUNK):
                    pt = psum.tile([128, CHUNK], F32, tag="pp")
                    for s in range(CHUNK // SEG):
                        nc.tensor.matmul(
                            pt[:, s * SEG : (s + 1) * SEG],
                            lhsT,
                            znT[c][:, s * SEG : (s + 1) * SEG],
                            start=True,
                            stop=True,
                        )
                    ej = junkp.tile([128, CHUNK], F32, tag="ej")
                    nc.scalar.activation(
                        ej, pt, AF.Exp, scale=INV_T, bias=b_neg10,
                        accum_out=acc[:, b, c : c + 1],
                    )
                    if c == 0:  # e_rr at cols b*128..+128 of chunk 0
                        scr = work.tile([128, 128], F32, tag="scr")
                        nc.vector.tensor_mul(scr, ej[:, b * 128 : b * 128 + 128], ident)
                        nc.vector.reduce_sum(e_diag[:, b : b + 1], scr, axis=AX.X)
                    if c == 2:  # e_pos at cols 4096 + b*128..+128
                        scr2 = work.tile([128, 128], F32, tag="scr2")
                        nc.vector.tensor_mul(scr2, ej[:, b * 128 : b * 128 + 128], ident)
                        nc.vector.reduce_sum(e_pos[:, b : b + 1], scr2, axis=AX.X)

            # ---- epilogue ------------------------------------------------
            sumexp = persist.tile([128, NBLK], F32)
            nc.vector.reduce_sum(sumexp, acc, axis=AX.X)      # [128,8,4] -> [128,8]
            S = persist.tile([128, NBLK], F32)
            nc.vector.tensor_sub(S, sumexp, e_diag)
            lnS = persist.tile([128, NBLK], F32)
            nc.scalar.activation(lnS, S, AF.Ln, bias=b_zero)
            lnp = persist.tile([128, NBLK], F32)
            nc.scalar.activation(lnp, e_pos, AF.Ln, bias=b_zero)
            contrib = persist.tile([128, NBLK], F32)
            nc.vector.tensor_sub(contrib, lnS, lnp)
            total = persist.tile([128, 1], F32)
            nc.vector.reduce_sum(total, contrib, axis=AX.X)
            nc.sync.dma_start(out=out[:, :], in_=total)

    nc.compile()
    return nc


_NC = None


def _get_nc() -> bass.Bass:
    global _NC
    if _NC is None:
        _NC = _build()
    return _NC


def kernel(emb0: np.ndarray, emb1: np.ndarray) -> np.ndarray:
    z = np.concatenate(
        [np.asarray(emb0, np.float32), np.asarray(emb1, np.float32)], axis=0
    )
    in_maps = [
        {"z": np.ascontiguousarray(np.roll(z, -c * ROWS, axis=0))}
        for c in range(NCORES)
    ]
    res = bass_utils.run_bass_kernel_spmd(_get_nc(), in_maps, core_ids=list(range(NCORES)))
    total = sum(float(r["partial"].sum(dtype=np.float64)) for r in res.results)
    return np.asarray(np.float32(total / N))
